# revision 32
# baseline (speedup 1.0000x reference)
"""Trainium2 Bass kernel for the sparse-attention ('interact' mask) transformer block.

Reference (B=4, N=1569, C=768, H=12, d=64, Dff=3072, F=9):
    h = LN(x)*g+b; qkv = h @ qkv_w.T; sparse attention (spatial rows attend
    only to the 9 temporal tokens, temporal rows attend to the 1560 spatial
    tokens, CLS also to itself); out = attn @ proj_w.T + proj_b;
    return out + MLP(LN(out)*g+b)

Sharding: 8 cores = 4 batches x 2 halves; local layout [780 spatial | 9
temporal | 1 zero pad] = 790 cols, feature-major [C, T] on chip.  Only
communication: pairwise AllReduce(add) of flash-style partials (l2, O2) for
the 9 temporal query rows, overlapped with the MLP GEMMs.

v2 design (vs. 479us baseline):
  - ALL matmul moving operands are bf16 (fp32r emitted fp32_mode=HIGH
    multi-pass: ~3x slower; measured).  LN g/b folded into weights/biases
    on the host so the device computes raw LN.
  - 2 token tiles (512+278) instead of 3 (512+268+10): the 10-col tile
    cost ~200ns/MM x 396 MMs = 83us of pure overhead.
  - weights resident/prefetched in an SBUF slot ring (no DMA serialization).
  - fc1+fc2 fused per hidden chunk (fc2 accumulates in 6 pinned PSUM banks)
    so no [128,T]x24 hid buffer and a dense warm PE stream.
  - second token tile of proj/LN2/MLP deferred behind the collective.
"""

import numpy as np
import sys
from contextlib import ExitStack

sys.path.insert(0, '/opt/trn_rl_repo')

import concourse.bass as bass
import concourse.bacc as bacc
import concourse.tile as tile
from concourse import mybir
from concourse.bass_utils import run_bass_kernel_spmd

# ---------------- problem constants ----------------
B, N, C = 4, 1569, 768
H, D = 12, 64
F = 9
DFF = 4 * C
NSP = N - F
SPH = NSP // 2
T = SPH + F + 1          # 790 local cols: [780 spatial | 9 temporal | 1 pad]
NCH = C // 128           # 6
NFF = DFF // 128         # 24
HF = H * F               # 108
NTB = (T + 127) // 128   # 7 token blocks (last = 22 rows)
SCALE = D ** -0.5

FP32 = mybir.dt.float32
BF16 = mybir.dt.bfloat16
AF = mybir.ActivationFunctionType
OP = mybir.AluOpType

TT = [(0, 512), (512, T)]        # main 2-tile split
TSP = [(0, 512), (512, SPH)]     # spatial-only (O1)


def build_kernel():
    nc = bacc.Bacc("TRN2", target_bir_lowering=False, debug=False,
                   num_devices=8)

    # ---------------- DRAM I/O ----------------
    xT = nc.dram_tensor("xT", [C, T], BF16, kind="ExternalInput")
    qkvWt = nc.dram_tensor("qkvWt", [C, 3 * C], BF16, kind="ExternalInput")
    projWt = nc.dram_tensor("projWt", [C, C], BF16, kind="ExternalInput")
    fc1Wt = nc.dram_tensor("fc1Wt", [C, DFF], BF16, kind="ExternalInput")
    fc2Wt = nc.dram_tensor("fc2Wt", [DFF, C], BF16, kind="ExternalInput")
    # packed constants: auxf [128,48] fp32 = biases (qk 0:12 | proj 12:18 |
    # fc1 18:42 | fc2 42:48); auxb [128,1253] bf16 = ones(0) | ident(1:129) |
    # e00(129:237) | bd9(237:249) | onesrow(249:377) | bd9T(377:485) |
    # vbrow(485:1253)
    auxf = nc.dram_tensor("auxf", [128, 48], FP32, kind="ExternalInput")
    auxb = nc.dram_tensor("auxb", [128, 1253], BF16, kind="ExternalInput")
    outT = nc.dram_tensor("outT", [C, T], FP32, kind="ExternalOutput")

    with tile.TileContext(nc) as tc, ExitStack() as ctx:
        cst = ctx.enter_context(tc.tile_pool(name="cst", bufs=1))
        wb = ctx.enter_context(tc.tile_pool(name="wb", bufs=12))
        wp = ctx.enter_context(tc.tile_pool(name="wp", bufs=6))
        xfp = ctx.enter_context(tc.tile_pool(name="xfp", bufs=6))
        aa = ctx.enter_context(tc.tile_pool(name="aa", bufs=18))
        sep = ctx.enter_context(tc.tile_pool(name="sep", bufs=1))
        sm = ctx.enter_context(tc.tile_pool(name="sm", bufs=1))
        ps = ctx.enter_context(tc.tile_pool(name="ps", bufs=1, space="PSUM"))
        dram = ctx.enter_context(tc.tile_pool(name="dram", bufs=1, space="DRAM"))

        # psum helper: rotate through mm(2) + acc0..5(1) for general groups
        ps_cycle = ['mm', 'acc0', 'mm', 'acc1', 'mm', 'acc2',
                    'mm', 'acc3', 'mm', 'acc4', 'mm', 'acc5']
        ps_i = [0]

        def pt(tag=None):
            if tag is None:
                tag = ps_cycle[ps_i[0] % len(ps_cycle)]
                ps_i[0] += 1
            return ps.tile([128, 512], FP32, tag=tag,
                           bufs=(2 if tag == 'mm' else 1), name="pst")

        # ---------------- x load (first on the sync queue, bf16) ----------
        xb = [aa.tile([128, T], BF16, tag="a", name=f"xb{ci}")
              for ci in range(NCH)]
        for ci in range(NCH):
            nc.sync.dma_start(xb[ci][:], xT[ci * 128:(ci + 1) * 128, :])

        # ---------------- packed constants (2 DMAs) ----------------
        auxf_t = cst.tile([128, 48], FP32, tag="auxf", name="auxf")
        nc.sync.dma_start(auxf_t[:], auxf[:])
        auxb_t = cst.tile([128, 1253], BF16, tag="auxb", name="auxb")
        nc.sync.dma_start(auxb_t[:], auxb[:])
        ones_t = auxb_t[:, 0:1]
        id_t = auxb_t[:, 1:129]
        e00_t = auxb_t[:, 129:237]
        bd9_t = auxb_t[:, 237:249]
        onesrow_t = auxb_t[:, 249:377]
        bd9T_t = auxb_t[:, 377:485]
        vbrow_t = auxb_t[:, 485:1253]
        qb_t = [auxf_t[:, j:j + 1] for j in range(12)]
        pb_t = [auxf_t[:, 12 + j:13 + j] for j in range(NCH)]
        f1b_t = [auxf_t[:, 18 + j:19 + j] for j in range(NFF)]
        f2b_t = [auxf_t[:, 42 + j:43 + j] for j in range(NCH)]

        # ---------------- weight prefetch ----------------
        # qkv q/k: 6 x [128,2304] in the big ring, column-group-ordered DMAs
        wqkv = [wb.tile([128, 3 * C], BF16, tag="w", name=f"wqkv{ci}",
                        padded_shape=[128, DFF]) for ci in range(NCH)]
        for cg in range(0, 3 * C, 512):
            gw = min(512, 3 * C - cg)
            for ci in range(NCH):
                nc.sync.dma_start(wqkv[ci][:, cg:cg + gw],
                                  qkvWt[ci * 128:(ci + 1) * 128, cg:cg + gw])
        wpj = [wp.tile([128, C], BF16, tag="wp", name=f"wpj{ci}")
               for ci in range(NCH)]
        for ci in range(NCH):
            nc.sync.dma_start(wpj[ci][:], projWt[ci * 128:(ci + 1) * 128, :])
        wf1 = [wb.tile([128, DFF], BF16, tag="w", name=f"wf1{ci}")
               for ci in range(NCH)]
        for ci in range(NCH):
            nc.sync.dma_start(wf1[ci][:], fc1Wt[ci * 128:(ci + 1) * 128, :])

        # ---------------- x^2 for LN1 stats ----------------
        sq = [aa.tile([128, T], BF16, tag="a", name=f"sq{ci}")
              for ci in range(NCH)]
        for ci in range(NCH):
            nc.scalar.activation(sq[ci][:], xb[ci][:], AF.Square)

        # ---------------- LN helpers ----------------
        def ln_stats(srcb, srcsq, t0, t1):
            w = t1 - t0
            p = pt()
            for ci in range(NCH):
                nc.tensor.matmul(p[0:1, :w], ones_t[:, 0:1],
                                 srcb[ci][:, t0:t1],
                                 start=(ci == 0), stop=(ci == NCH - 1))
            p2 = pt()
            for ci in range(NCH):
                nc.tensor.matmul(p2[0:1, :w], ones_t[:, 0:1],
                                 srcsq[ci][:, t0:t1],
                                 start=(ci == 0), stop=(ci == NCH - 1))
            return p, p2

        def ln_chain(p, p2, t0, t1):
            w = t1 - t0
            mu = sm.tile([1, 512], FP32, tag="mu", name="mu", bufs=1)
            tmp = sm.tile([1, 512], FP32, tag="tmp", name="tmp", bufs=1)
            al = sm.tile([1, 512], BF16, tag="al", name="al", bufs=1)
            be = sm.tile([1, 512], BF16, tag="be", name="be", bufs=1)
            nc.vector.tensor_scalar_mul(mu[0:1, :w], p[0:1, :w], 1.0 / C)
            nc.vector.tensor_mul(tmp[0:1, :w], mu[0:1, :w], mu[0:1, :w])
            # tmp = mu^2 - eps  so that  var+eps = ps2/C - tmp
            nc.vector.tensor_scalar_add(tmp[0:1, :w], tmp[0:1, :w], -1e-5)
            nc.vector.scalar_tensor_tensor(p2[0:1, :w], p2[0:1, :w], 1.0 / C,
                                           tmp[0:1, :w],
                                           op0=OP.mult, op1=OP.subtract)
            nc.scalar.activation(tmp[0:1, :w], p2[0:1, :w], AF.Sqrt)
            with nc.allow_low_precision(reason="bf16 LN scale intended"):
                nc.vector.reciprocal(al[0:1, :w], tmp[0:1, :w])
                nc.vector.scalar_tensor_tensor(be[0:1, :w], mu[0:1, :w],
                                               -1.0, al[0:1, :w],
                                               op0=OP.mult, op1=OP.mult)
            return al, be

        def ln_bcast(al, be, bcA, bcB, t0, t1):
            w = t1 - t0
            for src, dst in ((al, bcA), (be, bcB)):
                psb = pt(tag='mm')
                nc.tensor.matmul(psb[:, :w], onesrow_t[0:1, :],
                                 src[0:1, :w], start=True, stop=True)
                nc.scalar.copy(dst[:, t0:t1], psb[:, :w])

        def ln_apply(srcf, dst, bcA, bcB, t0, t1):
            # split chunks across DVE and GpSimd to halve the tail latency
            for ci in range(NCH):
                eng = nc.vector if ci % 2 == 0 else nc.gpsimd
                eng.tensor_mul(dst[ci][:, t0:t1], srcf[ci][:, t0:t1],
                               bcA[:, t0:t1])
                eng.tensor_tensor(dst[ci][:, t0:t1], dst[ci][:, t0:t1],
                                  bcB[:, t0:t1], op=OP.add)

        # ---------------- LN1 ----------------
        h = [aa.tile([128, T], BF16, tag="a", name=f"h{ci}")
             for ci in range(NCH)]
        bcA1 = sm.tile([128, T], BF16, tag="bc", name="bcA1", bufs=2)
        bcB1 = sm.tile([128, T], BF16, tag="bc", name="bcB1", bufs=2)

        s_t0, s2_t0 = ln_stats(xb, sq, *TT[0])
        al0, be0 = ln_chain(s_t0, s2_t0, *TT[0])
        ln_bcast(al0, be0, bcA1, bcB1, *TT[0])
        ln_apply(xb, h, bcA1, bcB1, *TT[0])
        s_t1, s2_t1 = ln_stats(xb, sq, *TT[1])
        al1, be1 = ln_chain(s_t1, s2_t1, *TT[1])

        # ---------------- qkv (q,k feature-major) ----------------
        k_t = [aa.tile([128, T], BF16, tag="a", name=f"k{ci}")
               for ci in range(NCH)]
        q_t = [aa.tile([128, T], BF16, tag="a", name=f"q{ci}")
               for ci in range(NCH)]

        def qk_group(j, t0, t1):
            # j in 0..11: 0-5 = q couts, 6-11 = k couts
            w = t1 - t0
            dst = q_t[j] if j < NCH else k_t[j - NCH]
            p = pt()
            for ci in range(NCH):
                nc.tensor.matmul(p[:, :w], wqkv[ci][:, j * 128:(j + 1) * 128],
                                 h[ci][:, t0:t1],
                                 start=(ci == 0), stop=(ci == NCH - 1))
            if j % 2 == 0:
                nc.scalar.activation(dst[:, t0:t1], p[:, :w], AF.Identity,
                                     bias=qb_t[j][:, 0:1])
            else:
                nc.vector.tensor_scalar_add(dst[:, t0:t1], p[:, :w],
                                            qb_t[j][:, 0:1])

        # k first, then q-t1 (what the collective-critical p2 path needs);
        # q-t0 is deferred until after the collective launches
        for j in range(6, 9):
            qk_group(j, *TT[0])
        # t1 broadcast + apply overlap the first qkv groups
        ln_bcast(al1, be1, bcA1, bcB1, *TT[1])
        ln_apply(xb, h, bcA1, bcB1, *TT[1])
        for j in range(9, 12):
            qk_group(j, *TT[0])
        for j in range(6, 12):
            qk_group(j, *TT[1])
        for j in range(6):
            qk_group(j, *TT[1])

        # v bias broadcast [128, C]
        vb_bc = sm.tile([128, C], BF16, tag="vbbc", name="vb_bc")
        for cg in range(0, C, 512):
            gw = min(512, C - cg)
            psb = pt()
            nc.tensor.matmul(psb[:, :gw], onesrow_t[0:1, :],
                             vbrow_t[0:1, cg:cg + gw], start=True, stop=True)
            nc.scalar.copy(vb_bc[:, cg:cg + gw], psb[:, :gw])

        # v token-major [T, C]
        v_t = [sep.tile([128, C], BF16, tag="v", name=f"v{tb}", bufs=NTB)
               for tb in range(NTB)]
        for tb in range(NTB):
            p0, p1_ = tb * 128, min((tb + 1) * 128, T)
            pp = p1_ - p0
            for cg in range(0, C, 512):
                gw = min(512, C - cg)
                p = pt()
                for ci in range(NCH):
                    nc.tensor.matmul(p[:pp, :gw], h[ci][:, p0:p1_],
                                     wqkv[ci][:, 2 * C + cg:2 * C + cg + gw],
                                     start=(ci == 0), stop=(ci == NCH - 1))
                nc.vector.tensor_tensor(v_t[tb][:pp, cg:cg + gw], p[:pp, :gw],
                                        vb_bc[:pp, cg:cg + gw], op=OP.add)

        # =========================================================
        # sparse attention — ordered so the PE stream stays dense and the
        # collective launches right after O2
        # =========================================================
        attnout = [sep.tile([128, T], BF16, tag="ao", name=f"ao{ci}", bufs=6)
                   for ci in range(NCH)]
        for ci in range(NCH):
            # zero pad col 789 (and 788, rewritten by the temporal patch)
            nc.vector.memzero(attnout[ci][:, T - 2:T])

        # kbd (h,j) cols / qbd (j,h) cols, block-diag by head
        kbd = [sm.tile([128, HF], BF16, tag=f"kbd{ci}", name=f"kbd{ci}")
               for ci in range(NCH)]
        qbd = [sm.tile([128, HF], BF16, tag=f"qbd{ci}", name=f"qbd{ci}")
               for ci in range(NCH)]
        for ci in range(NCH):
            nc.vector.memzero(kbd[ci][:])
            nc.vector.memzero(qbd[ci][:])
        for hh in range(H):
            ci, po = hh // 2, (hh % 2) * 64
            nc.vector.tensor_copy(qbd[ci][po:po + 64, hh:hh + 97:H],
                                  q_t[ci][po:po + 64, SPH:SPH + F])
            nc.vector.tensor_copy(kbd[ci][po:po + 64, hh * F:(hh + 1) * F],
                                  k_t[ci][po:po + 64, SPH:SPH + F])

        # rest of qkv: q @ t0
        for j in range(6):
            qk_group(j, *TT[0])

        # vtmp_bd [108, C]: rows (h,j) = temporal v of head h at cols h*64..
        vtmp_bd = sm.tile([HF, C], BF16, tag="vtmpbd", name="vtmpbd")
        nc.vector.memzero(vtmp_bd[0:HF, :])
        for hh in range(H):
            nc.sync.dma_start(vtmp_bd[hh * F:(hh + 1) * F,
                                      hh * 64:(hh + 1) * 64],
                              v_t[6][12:12 + F, hh * 64:(hh + 1) * 64])

        # S1/P1: all local queries vs 9 temporal keys -> p1 [108, T]
        p1 = sm.tile([HF, T], BF16, tag="p1", name="p1")
        for (t0, t1) in TT:
            w = t1 - t0
            p = pt()
            for ci in range(NCH):
                nc.tensor.matmul(p[0:HF, :w], kbd[ci][:], q_t[ci][:, t0:t1],
                                 start=(ci == 0), stop=(ci == NCH - 1))
            nc.scalar.activation(p1[0:HF, t0:t1], p[0:HF, :w], AF.Exp,
                                 scale=SCALE)

        # S2T/P2T: temporal queries vs all local keys, token-major [T, 108]
        p2 = [sm.tile([128, HF], BF16, tag="p2", name=f"p2{tb}", bufs=NTB)
              for tb in range(NTB)]
        for tb in range(NTB):
            p0, p1_ = tb * 128, min((tb + 1) * 128, T)
            pp = p1_ - p0
            p = pt()
            for ci in range(NCH):
                nc.tensor.matmul(p[:pp, 0:HF], k_t[ci][:, p0:p1_], qbd[ci][:],
                                 start=(ci == 0), stop=(ci == NCH - 1))
            nc.scalar.activation(p2[tb][:pp, :], p[:pp, 0:HF], AF.Exp,
                                 scale=SCALE)

        # lsp[h,t] = sum_j p1[(h,j),t]; rlsp = 1/lsp (bf16)
        rlsp = sm.tile([H, T], BF16, tag="rlsp", name="rlsp")
        for (t0, t1) in TT:
            w = t1 - t0
            p = pt()
            nc.tensor.matmul(p[0:H, :w], bd9_t[0:HF, :], p1[0:HF, t0:t1],
                             start=True, stop=True)
            with nc.allow_low_precision(reason="bf16 softmax recip intended"):
                nc.vector.reciprocal(rlsp[0:H, t0:t1], p[0:H, :w])

        # rlsp9 [108, T] = rlsp repeated per j; p1 *= rlsp9 (pre-normalize)
        rlsp9 = sm.tile([HF, T], BF16, tag="rlsp9", name="rlsp9")
        for (t0, t1) in TT:
            w = t1 - t0
            p = pt()
            nc.tensor.matmul(p[0:HF, :w], bd9T_t[0:H, :], rlsp[0:H, t0:t1],
                             start=True, stop=True)
            nc.vector.tensor_copy(rlsp9[0:HF, t0:t1], p[0:HF, :w])

        # mask token-block 6: rows 0-11 (spatial) pass, row 12 (CLS key)
        # kept only for q_j=0 on even cores, rows 13-21 (temporal+pad) zeroed
        nc.vector.tensor_mul(p2[6][0:22, :], p2[6][0:22, :], e00_t[0:22, :])

        # l2 partial [1,108]
        l2row = sm.tile([1, HF], FP32, tag="l2", name="l2row")
        p_l2 = pt()
        for tb in range(NTB):
            p0, p1_ = tb * 128, min((tb + 1) * 128, T)
            pp = p1_ - p0
            nc.tensor.matmul(p_l2[0:1, 0:HF], ones_t[:pp, 0:1],
                             p2[tb][:pp, :],
                             start=(tb == 0), stop=(tb == NTB - 1))
        nc.scalar.copy(l2row[:], p_l2[0:1, 0:HF])

        # normalize p1 per tile (DVE, overlaps O2 matmuls)
        for (t0, t1) in TT:
            nc.vector.tensor_mul(p1[0:HF, t0:t1], p1[0:HF, t0:t1],
                                 rlsp9[0:HF, t0:t1])

        # O2 partial [9, C]
        o2 = sm.tile([F, C], FP32, tag="o2", name="o2")
        for hh in range(H):
            p = pt()
            for tb in range(NTB):
                p0, p1_ = tb * 128, min((tb + 1) * 128, T)
                pp = p1_ - p0
                nc.tensor.matmul(p[0:F, 0:64],
                                 p2[tb][:pp, hh:hh + 97:H],
                                 v_t[tb][:pp, hh * 64:(hh + 1) * 64],
                                 start=(tb == 0), stop=(tb == NTB - 1))
            nc.scalar.copy(o2[0:F, hh * 64:(hh + 1) * 64], p[0:F, 0:64])

        # pairwise AllReduce of (o2 | l2) in one [10, C] buffer — launched
        # as early as possible; consumed in the fused-t0 mid hook
        cc_in = dram.tile([F + 1, C], FP32, tag="cc_in", name="cc_in")
        cc_out = dram.tile([F + 1, C], FP32, tag="cc_out", name="cc_out")
        groups = [[0, 1], [2, 3], [4, 5], [6, 7]]
        nc.scalar.dma_start(cc_in[0:F, :], o2[0:F, :])
        nc.scalar.dma_start(cc_in[F:F + 1, 0:HF], l2row[:])
        nc.gpsimd.collective_compute("AllReduce", mybir.AluOpType.add,
                                     replica_groups=groups,
                                     ins=[cc_in.opt()], outs=[cc_out.opt()])

        # O1: spatial attention out (fills the post-launch PE slot)
        for ci in range(NCH):
            for (t0, t1) in TSP:
                w = t1 - t0
                p = pt()
                nc.tensor.matmul(p[:, :w],
                                 vtmp_bd[0:HF, ci * 128:(ci + 1) * 128],
                                 p1[0:HF, t0:t1], start=True, stop=True)
                nc.vector.tensor_copy(attnout[ci][:, t0:t1], p[:, :w])

        # ---------------- fc2 weights into recycled qkv slots ----------------
        w2g = [wb.tile([128, DFF], BF16, tag="w", name=f"w2g{g}")
               for g in range(NCH)]
        for g in range(NCH):
            for kk in range(4):
                cchunk = 4 * g + kk
                nc.sync.dma_start(w2g[g][:, kk * C:(kk + 1) * C],
                                  fc2Wt[cchunk * 128:(cchunk + 1) * 128, :])

        # =========================================================
        # proj t0 -> LN2 t0 -> fused fc1+fc2 t0 (collective overlapped)
        # =========================================================
        projout = [xfp.tile([128, T], FP32, tag="xf", name=f"po{ci}")
                   for ci in range(NCH)]
        pb = [sep.tile([128, T], BF16, tag="pbb", name=f"pbb{ci}", bufs=6)
              for ci in range(NCH)]
        sq2 = [aa.tile([128, T], BF16, tag="a", name=f"sq2{ci}")
               for ci in range(NCH)]
        h2 = [sep.tile([128, T], BF16, tag="h2", name=f"h2{ci}", bufs=6)
              for ci in range(NCH)]
        bcA2 = sm.tile([128, T], BF16, tag="bc", name="bcA2", bufs=2)
        bcB2 = sm.tile([128, T], BF16, tag="bc", name="bcB2", bufs=2)

        def proj_tile(t0, t1, stats_tags=None):
            # proj couts; optionally interleave LN2 stats accumulation MMs
            # (stats_tags name two free PSUM banks to pin for the sweep)
            w = t1 - t0
            sA = sB = None
            if stats_tags:
                sA, sB = pt(tag=stats_tags[0]), pt(tag=stats_tags[1])
            for j in range(NCH):
                p = pt(tag='mm')
                for ci in range(NCH):
                    nc.tensor.matmul(p[:, :w],
                                     wpj[ci][:, j * 128:(j + 1) * 128],
                                     attnout[ci][:, t0:t1],
                                     start=(ci == 0), stop=(ci == NCH - 1))
                nc.scalar.activation(projout[j][:, t0:t1], p[:, :w],
                                     AF.Identity, bias=pb_t[j][:, 0:1])
                nc.vector.tensor_scalar_add(pb[j][:, t0:t1], p[:, :w],
                                            pb_t[j][:, 0:1])
                nc.gpsimd.tensor_mul(sq2[j][:, t0:t1], pb[j][:, t0:t1],
                                     pb[j][:, t0:t1])
                if stats_tags:
                    nc.tensor.matmul(sA[0:1, :w], ones_t[:, 0:1],
                                     pb[j][:, t0:t1], start=(j == 0),
                                     stop=(j == NCH - 1),
                                     skip_group_check=True)
                    nc.tensor.matmul(sB[0:1, :w], ones_t[:, 0:1],
                                     sq2[j][:, t0:t1], start=(j == 0),
                                     stop=(j == NCH - 1),
                                     skip_group_check=True)
            return sA, sB

        def ln2_finish(sA, sB, t0, t1):
            al2, be2 = ln_chain(sA, sB, t0, t1)
            ln_bcast(al2, be2, bcA2, bcB2, t0, t1)
            ln_apply(projout, h2, bcA2, bcB2, t0, t1)

        def ln2_stats_seq(t0, t1):
            w = t1 - t0
            p = pt(tag='mm')
            for ci in range(NCH):
                nc.tensor.matmul(p[0:1, :w], ones_t[:, 0:1], pb[ci][:, t0:t1],
                                 start=(ci == 0), stop=(ci == NCH - 1))
            p2_ = pt(tag='mm')
            for ci in range(NCH):
                nc.tensor.matmul(p2_[0:1, :w], ones_t[:, 0:1],
                                 sq2[ci][:, t0:t1],
                                 start=(ci == 0), stop=(ci == NCH - 1))
            return p, p2_

        sA0, sB0 = proj_tile(*TT[0], stats_tags=('acc0', 'acc1'))
        ln2_finish(sA0, sB0, *TT[0])

        # fused fc1+fc2: fc2 accumulates into 6 pinned PSUM banks
        def fused(t0, t1, mid_hook=None):
            w = t1 - t0
            acc = [pt(tag=f'acc{cb}') for cb in range(NCH)]
            for g in range(NFF):
                if mid_hook is not None and g == 12:
                    mid_hook()
                pf = pt(tag='mm')
                for ci in range(NCH):
                    nc.tensor.matmul(pf[:, :w],
                                     wf1[ci][:, g * 128:(g + 1) * 128],
                                     h2[ci][:, t0:t1],
                                     start=(ci == 0), stop=(ci == NCH - 1))
                hidt = sm.tile([128, 512], BF16, tag="hid", name=f"hid{g}",
                               bufs=4)
                nc.scalar.activation(hidt[:, :w], pf[:, :w], AF.Gelu,
                                     bias=f1b_t[g][:, 0:1])
                wg, kk = g // 4, g % 4
                for cb in range(NCH):
                    nc.tensor.matmul(acc[cb][:, :w],
                                     w2g[wg][:, kk * C + cb * 128:
                                             kk * C + (cb + 1) * 128],
                                     hidt[:, :w],
                                     start=(g == 0), stop=(g == NFF - 1),
                                     skip_group_check=True)
            for cb in range(NCH):
                st = sm.tile([128, 512], FP32, tag="st", name=f"st{cb}",
                             bufs=4)
                nc.vector.scalar_tensor_tensor(st[:, :w], acc[cb][:, :w],
                                               f2b_t[cb][:, 0:1],
                                               projout[cb][:, t0:t1],
                                               op0=OP.add, op1=OP.add)
                nc.sync.dma_start(outT[cb * 128:(cb + 1) * 128, t0:t1],
                                  st[:, :w])

        # deferred: collective landing -> temporal cols -> t1 of everything
        def temporal_patch():
            l2jh = sm.tile([F, H], FP32, tag="l2jh", name="l2jh")
            o2n = sm.tile([F, C], BF16, tag="o2n", name="o2n")
            # SWDGE cast-DMA f32 -> bf16 straight into o2n
            nc.gpsimd.dma_start(o2n[0:F, :], cc_out[0:F, :])
            # [1,108] DRAM row -> [9,12] SBUF in one reshaping DMA
            nc.scalar.dma_start(l2jh[0:F, :], cc_out[F:F + 1, 0:HF])
            nc.vector.reciprocal(l2jh[0:F, :], l2jh[0:F, :])
            for hh in range(H):
                nc.vector.tensor_scalar_mul(o2n[0:F, hh * 64:(hh + 1) * 64],
                                            o2n[0:F, hh * 64:(hh + 1) * 64],
                                            l2jh[0:F, hh:hh + 1])
            for ci in range(NCH):
                p = ps.tile([128, 512], BF16, tag='mm', bufs=2, name="pstb")
                nc.tensor.transpose(p[:, 0:F],
                                    o2n[0:F, ci * 128:(ci + 1) * 128],
                                    id_t[0:F, 0:F])
                nc.scalar.copy(attnout[ci][:, SPH:SPH + F], p[:, 0:F])
            proj_tile(*TT[1])
            sA1, sB1 = ln2_stats_seq(*TT[1])
            ln2_finish(sA1, sB1, *TT[1])

        fused(*TT[0], mid_hook=temporal_patch)
        fused(*TT[1])

    nc.compile()
    return nc


# ---------------- host side ----------------
_compiled = {}


def kernel(**inputs):
    x = np.ascontiguousarray(np.asarray(inputs['x'], np.float32))
    qkv_w = np.asarray(inputs['qkv_w'], np.float32)
    proj_w = np.asarray(inputs['proj_w'], np.float32)
    proj_b = np.asarray(inputs['proj_b'], np.float32)
    fc1_w = np.asarray(inputs['fc1_w'], np.float32)
    fc1_b = np.asarray(inputs['fc1_b'], np.float32)
    fc2_w = np.asarray(inputs['fc2_w'], np.float32)
    fc2_b = np.asarray(inputs['fc2_b'], np.float32)
    g = np.asarray(inputs['ln2_g'], np.float32)
    bb = np.asarray(inputs['ln2_b'], np.float32)

    import ml_dtypes
    bf16 = ml_dtypes.bfloat16

    # fold LN affine (g, b) into the consuming GEMMs:
    #   W @ (LNraw(x)*g + b) = (W*g) @ LNraw(x) + W@b
    qkvW = qkv_w * g[None, :]                 # [3C, C]
    qkvB = qkv_w @ bb                         # [3C]
    fc1W = fc1_w * g[None, :]
    fc1Bf = fc1_b + fc1_w @ bb

    qkvWt = np.ascontiguousarray(qkvW.T).astype(bf16)     # [C, 3C]
    projWt = np.ascontiguousarray(proj_w.T).astype(bf16)  # [C, C]
    fc1Wt = np.ascontiguousarray(fc1W.T).astype(bf16)     # [C, DFF]
    fc2Wt = np.ascontiguousarray(fc2_w.T).astype(bf16)    # [DFF, C]

    # packed fp32 biases [128, 48]
    auxf_np = np.zeros((128, 48), np.float32)
    for j in range(12):
        auxf_np[:, j] = qkvB[j * 128:(j + 1) * 128]
    for j in range(6):
        auxf_np[:, 12 + j] = proj_b[j * 128:(j + 1) * 128]
    for j in range(24):
        auxf_np[:, 18 + j] = fc1Bf[j * 128:(j + 1) * 128]
    for j in range(6):
        auxf_np[:, 42 + j] = fc2_b[j * 128:(j + 1) * 128]

    # packed bf16 constants [128, 1253]:
    # ones(0) | ident(1:129) | e00(129:237) | bd9(237:249) |
    # onesrow(249:377) | bd9T(377:485) | vbrow(485:1253)
    bd9_np = np.zeros((H * F, H), np.float32)
    for hh in range(H):
        bd9_np[hh * F:(hh + 1) * F, hh] = 1.0
    auxb_np = np.zeros((128, 1253), np.float32)
    auxb_np[:, 0] = 1.0
    auxb_np[:, 1:129] = np.eye(128)
    # e00: multiplicative mask for p2 token-block 6 (local tokens 768..789):
    # rows 0-11 = spatial -> 1; row 12 = CLS key -> keep only q_j=0 cols
    # (cols 0..11 in (j,h) order) on even cores; rows 13-21 -> 0
    auxb_np[0:12, 129:237] = 1.0
    auxb_np[0:108, 237:249] = bd9_np
    auxb_np[0, 249:377] = 1.0
    auxb_np[0:12, 377:485] = bd9_np.T
    auxb_np[0, 485:1253] = qkvB[2 * C:]
    auxb_even = auxb_np.copy()
    auxb_even[12, 129 + 0:129 + H] = 1.0    # CLS self-term on even cores

    in_maps = []
    for core in range(8):
        b_, half = core // 2, core % 2
        sp = x[b_, F + half * SPH: F + (half + 1) * SPH]     # [780, C]
        tmp = x[b_, 0:F]                                     # [9, C]
        pad = np.zeros((1, C), np.float32)
        xTn = np.ascontiguousarray(
            np.concatenate([sp, tmp, pad], 0).T).astype(bf16)  # [C, 790]
        in_maps.append(dict(
            xT=xTn, qkvWt=qkvWt, projWt=projWt, fc1Wt=fc1Wt, fc2Wt=fc2Wt,
            auxf=auxf_np,
            auxb=(auxb_even if half == 0 else auxb_np).astype(bf16)))

    if 'nc' not in _compiled:
        _compiled['nc'] = build_kernel()
    nc = _compiled['nc']
    res = run_bass_kernel_spmd(nc, in_maps, list(range(8)))
    _compiled['last_result'] = res

    out = np.zeros((B, N, C), np.float32)
    for core in range(8):
        b_, half = core // 2, core % 2
        oT = res.results[core]['outT']                       # [C, 790]
        if half == 0:
            out[b_, 0:F] = oT[:, SPH:SPH + F].T
            out[b_, F:F + SPH] = oT[:, 0:SPH].T
        else:
            out[b_, F + SPH:N] = oT[:, 0:SPH].T
    return out


if __name__ == '__main__':
    from reference import setup_inputs, reference
    inputs = {k: np.asarray(v) for k, v in setup_inputs().items()}
    out = kernel(**inputs)
    print("kernel ran, out shape", out.shape)


# revision 34
# speedup vs baseline: 1.3012x; 1.3012x over previous
"""Trainium2 Bass kernel for the sparse-attention ('interact' mask) transformer block.

Reference (B=4, N=1569, C=768, H=12, d=64, Dff=3072, F=9):
    h = LN(x)*g+b; qkv = h @ qkv_w.T; sparse attention (spatial rows attend
    only to the 9 temporal tokens, temporal rows attend to the 1560 spatial
    tokens, CLS also to itself); out = attn @ proj_w.T + proj_b;
    return out + MLP(LN(out)*g+b)

Sharding: 8 cores = 4 batches x 2 halves; local layout [780 spatial | 9
temporal | 1 zero pad] = 790 cols, feature-major [C, T] on chip.  Only
communication: pairwise AllReduce(add) of flash-style partials (l2, O2) for
the 9 temporal query rows, overlapped with the MLP GEMMs.

v2 design (vs. 479us baseline):
  - ALL matmul moving operands are bf16 (fp32r emitted fp32_mode=HIGH
    multi-pass: ~3x slower; measured).  LN g/b folded into weights/biases
    on the host so the device computes raw LN.
  - 2 token tiles (512+278) instead of 3 (512+268+10): the 10-col tile
    cost ~200ns/MM x 396 MMs = 83us of pure overhead.
  - weights resident/prefetched in an SBUF slot ring (no DMA serialization).
  - fc1+fc2 fused per hidden chunk (fc2 accumulates in 6 pinned PSUM banks)
    so no [128,T]x24 hid buffer and a dense warm PE stream.
  - second token tile of proj/LN2/MLP deferred behind the collective.
"""

import numpy as np
import sys
from contextlib import ExitStack

sys.path.insert(0, '/opt/trn_rl_repo')

import concourse.bass as bass
import concourse.bacc as bacc
import concourse.tile as tile
from concourse import mybir
from concourse.bass_utils import run_bass_kernel_spmd

# ---------------- problem constants ----------------
B, N, C = 4, 1569, 768
H, D = 12, 64
F = 9
DFF = 4 * C
NSP = N - F
SPH = NSP // 2
T = SPH + F + 1          # 790 local cols: [780 spatial | 9 temporal | 1 pad]
NCH = C // 128           # 6
NFF = DFF // 128         # 24
HF = H * F               # 108
NTB = (T + 127) // 128   # 7 token blocks (last = 22 rows)
SCALE = D ** -0.5

FP32 = mybir.dt.float32
BF16 = mybir.dt.bfloat16
AF = mybir.ActivationFunctionType
OP = mybir.AluOpType

TT = [(0, 512), (512, T)]        # main 2-tile split
TSP = [(0, 512), (512, SPH)]     # spatial-only (O1)


def build_kernel():
    nc = bacc.Bacc("TRN2", target_bir_lowering=False, debug=False,
                   num_devices=8)

    # ---------------- DRAM I/O ----------------
    xT = nc.dram_tensor("xT", [C, T], BF16, kind="ExternalInput")
    qkvWt = nc.dram_tensor("qkvWt", [C, 3 * C], BF16, kind="ExternalInput")
    projWt = nc.dram_tensor("projWt", [C, C], BF16, kind="ExternalInput")
    fc1Wt = nc.dram_tensor("fc1Wt", [C, DFF], BF16, kind="ExternalInput")
    fc2Wt = nc.dram_tensor("fc2Wt", [DFF, C], BF16, kind="ExternalInput")
    # packed constants: auxf [128,48] fp32 = biases (qk 0:12 | proj 12:18 |
    # fc1 18:42 | fc2 42:48); auxb [128,1253] bf16 = ones(0) | ident(1:129) |
    # e00(129:237) | bd9(237:249) | onesrow(249:377) | bd9T(377:485) |
    # vbrow(485:1253)
    auxf = nc.dram_tensor("auxf", [128, 48], FP32, kind="ExternalInput")
    auxb = nc.dram_tensor("auxb", [128, 1253], BF16, kind="ExternalInput")
    outT = nc.dram_tensor("outT", [C, T], FP32, kind="ExternalOutput")

    with tile.TileContext(nc) as tc, ExitStack() as ctx:
        cst = ctx.enter_context(tc.tile_pool(name="cst", bufs=1))
        wb = ctx.enter_context(tc.tile_pool(name="wb", bufs=12))
        wp = ctx.enter_context(tc.tile_pool(name="wp", bufs=6))
        xfp = ctx.enter_context(tc.tile_pool(name="xfp", bufs=6))
        aa = ctx.enter_context(tc.tile_pool(name="aa", bufs=18))
        sep = ctx.enter_context(tc.tile_pool(name="sep", bufs=1))
        sm = ctx.enter_context(tc.tile_pool(name="sm", bufs=1))
        ps = ctx.enter_context(tc.tile_pool(name="ps", bufs=1, space="PSUM"))
        dram = ctx.enter_context(tc.tile_pool(name="dram", bufs=1, space="DRAM"))

        # psum helper: rotate through mm(2) + acc0..5(1) for general groups
        ps_cycle = ['mm', 'acc0', 'mm', 'acc1', 'mm', 'acc2',
                    'mm', 'acc3', 'mm', 'acc4', 'mm', 'acc5']
        ps_i = [0]

        def pt(tag=None):
            if tag is None:
                tag = ps_cycle[ps_i[0] % len(ps_cycle)]
                ps_i[0] += 1
            return ps.tile([128, 512], FP32, tag=tag,
                           bufs=(2 if tag == 'mm' else 1), name="pst")

        # ---------------- x load (first on the sync queue, bf16) ----------
        xb = [aa.tile([128, T], BF16, tag="a", name=f"xb{ci}")
              for ci in range(NCH)]
        for ci in range(NCH):
            nc.sync.dma_start(xb[ci][:], xT[ci * 128:(ci + 1) * 128, :])

        # ---------------- packed constants (2 DMAs) ----------------
        auxf_t = cst.tile([128, 48], FP32, tag="auxf", name="auxf")
        nc.sync.dma_start(auxf_t[:], auxf[:])
        auxb_t = cst.tile([128, 1253], BF16, tag="auxb", name="auxb")
        nc.sync.dma_start(auxb_t[:], auxb[:])
        ones_t = auxb_t[:, 0:1]
        id_t = auxb_t[:, 1:129]
        e00_t = auxb_t[:, 129:237]
        bd9_t = auxb_t[:, 237:249]
        onesrow_t = auxb_t[:, 249:377]
        bd9T_t = auxb_t[:, 377:485]
        vbrow_t = auxb_t[:, 485:1253]
        qb_t = [auxf_t[:, j:j + 1] for j in range(12)]
        pb_t = [auxf_t[:, 12 + j:13 + j] for j in range(NCH)]
        f1b_t = [auxf_t[:, 18 + j:19 + j] for j in range(NFF)]
        f2b_t = [auxf_t[:, 42 + j:43 + j] for j in range(NCH)]

        # ---------------- weight prefetch ----------------
        # qkv q/k: 6 x [128,2304] in the big ring, column-group-ordered DMAs
        wqkv = [wb.tile([128, 3 * C], BF16, tag="w", name=f"wqkv{ci}",
                        padded_shape=[128, DFF]) for ci in range(NCH)]
        for cg in range(0, 3 * C, 512):
            gw = min(512, 3 * C - cg)
            for ci in range(NCH):
                nc.sync.dma_start(wqkv[ci][:, cg:cg + gw],
                                  qkvWt[ci * 128:(ci + 1) * 128, cg:cg + gw])
        wpj = [wp.tile([128, C], BF16, tag="wp", name=f"wpj{ci}")
               for ci in range(NCH)]
        for ci in range(NCH):
            nc.sync.dma_start(wpj[ci][:], projWt[ci * 128:(ci + 1) * 128, :])
        wf1 = [wb.tile([128, DFF], BF16, tag="w", name=f"wf1{ci}")
               for ci in range(NCH)]
        for ci in range(NCH):
            nc.sync.dma_start(wf1[ci][:], fc1Wt[ci * 128:(ci + 1) * 128, :])

        # ---------------- x^2 for LN1 stats ----------------
        sq = [aa.tile([128, T], BF16, tag="a", name=f"sq{ci}")
              for ci in range(NCH)]
        for ci in range(NCH):
            nc.scalar.activation(sq[ci][:], xb[ci][:], AF.Square)

        # ---------------- LN helpers ----------------
        def ln_stats(srcb, srcsq, t0, t1):
            w = t1 - t0
            p = pt()
            for ci in range(NCH):
                nc.tensor.matmul(p[0:1, :w], ones_t[:, 0:1],
                                 srcb[ci][:, t0:t1],
                                 start=(ci == 0), stop=(ci == NCH - 1))
            p2 = pt()
            for ci in range(NCH):
                nc.tensor.matmul(p2[0:1, :w], ones_t[:, 0:1],
                                 srcsq[ci][:, t0:t1],
                                 start=(ci == 0), stop=(ci == NCH - 1))
            return p, p2

        def ln_chain(p, p2, t0, t1):
            w = t1 - t0
            mu = sm.tile([1, 512], FP32, tag="mu", name="mu", bufs=1)
            tmp = sm.tile([1, 512], FP32, tag="tmp", name="tmp", bufs=1)
            al = sm.tile([1, 512], BF16, tag="al", name="al", bufs=1)
            be = sm.tile([1, 512], BF16, tag="be", name="be", bufs=1)
            nc.vector.tensor_scalar_mul(mu[0:1, :w], p[0:1, :w], 1.0 / C)
            nc.vector.tensor_mul(tmp[0:1, :w], mu[0:1, :w], mu[0:1, :w])
            # tmp = mu^2 - eps  so that  var+eps = ps2/C - tmp
            nc.vector.tensor_scalar_add(tmp[0:1, :w], tmp[0:1, :w], -1e-5)
            nc.vector.scalar_tensor_tensor(p2[0:1, :w], p2[0:1, :w], 1.0 / C,
                                           tmp[0:1, :w],
                                           op0=OP.mult, op1=OP.subtract)
            nc.scalar.activation(tmp[0:1, :w], p2[0:1, :w], AF.Sqrt)
            with nc.allow_low_precision(reason="bf16 LN scale intended"):
                nc.vector.reciprocal(al[0:1, :w], tmp[0:1, :w])
                nc.vector.scalar_tensor_tensor(be[0:1, :w], mu[0:1, :w],
                                               -1.0, al[0:1, :w],
                                               op0=OP.mult, op1=OP.mult)
            return al, be

        def ln_bcast(al, be, bcA, bcB, t0, t1):
            w = t1 - t0
            for src, dst in ((al, bcA), (be, bcB)):
                psb = pt(tag='mm')
                nc.tensor.matmul(psb[:, :w], onesrow_t[0:1, :],
                                 src[0:1, :w], start=True, stop=True)
                nc.scalar.copy(dst[:, t0:t1], psb[:, :w])

        def ln_apply(srcf, dst, bcA, bcB, t0, t1):
            for ci in range(NCH):
                nc.vector.tensor_mul(dst[ci][:, t0:t1], srcf[ci][:, t0:t1],
                                     bcA[:, t0:t1])
                nc.vector.tensor_tensor(dst[ci][:, t0:t1], dst[ci][:, t0:t1],
                                        bcB[:, t0:t1], op=OP.add)

        # ---------------- LN1 ----------------
        h = [aa.tile([128, T], BF16, tag="a", name=f"h{ci}")
             for ci in range(NCH)]
        bcA1 = sm.tile([128, T], BF16, tag="bc", name="bcA1", bufs=2)
        bcB1 = sm.tile([128, T], BF16, tag="bc", name="bcB1", bufs=2)

        s_t0, s2_t0 = ln_stats(xb, sq, *TT[0])
        al0, be0 = ln_chain(s_t0, s2_t0, *TT[0])
        ln_bcast(al0, be0, bcA1, bcB1, *TT[0])
        ln_apply(xb, h, bcA1, bcB1, *TT[0])
        s_t1, s2_t1 = ln_stats(xb, sq, *TT[1])
        al1, be1 = ln_chain(s_t1, s2_t1, *TT[1])

        # ---------------- qkv (q,k feature-major) ----------------
        k_t = [aa.tile([128, T], BF16, tag="a", name=f"k{ci}")
               for ci in range(NCH)]
        q_t = [aa.tile([128, T], BF16, tag="a", name=f"q{ci}")
               for ci in range(NCH)]

        def qk_group(j, t0, t1):
            # j in 0..11: 0-5 = q couts, 6-11 = k couts
            w = t1 - t0
            dst = q_t[j] if j < NCH else k_t[j - NCH]
            p = pt()
            for ci in range(NCH):
                nc.tensor.matmul(p[:, :w], wqkv[ci][:, j * 128:(j + 1) * 128],
                                 h[ci][:, t0:t1],
                                 start=(ci == 0), stop=(ci == NCH - 1))
            if j % 2 == 0:
                nc.scalar.activation(dst[:, t0:t1], p[:, :w], AF.Identity,
                                     bias=qb_t[j][:, 0:1])
            else:
                nc.vector.tensor_scalar_add(dst[:, t0:t1], p[:, :w],
                                            qb_t[j][:, 0:1])

        # k first, then q-t1 (what the collective-critical p2 path needs);
        # q-t0 is deferred until after the collective launches
        for j in range(6, 9):
            qk_group(j, *TT[0])
        # t1 broadcast + apply overlap the first qkv groups
        ln_bcast(al1, be1, bcA1, bcB1, *TT[1])
        ln_apply(xb, h, bcA1, bcB1, *TT[1])
        for j in range(9, 12):
            qk_group(j, *TT[0])
        for j in range(6, 12):
            qk_group(j, *TT[1])
        for j in range(6):
            qk_group(j, *TT[1])

        # v bias broadcast [128, C]
        vb_bc = sm.tile([128, C], BF16, tag="vbbc", name="vb_bc")
        for cg in range(0, C, 512):
            gw = min(512, C - cg)
            psb = pt()
            nc.tensor.matmul(psb[:, :gw], onesrow_t[0:1, :],
                             vbrow_t[0:1, cg:cg + gw], start=True, stop=True)
            nc.scalar.copy(vb_bc[:, cg:cg + gw], psb[:, :gw])

        # v token-major [T, C]
        v_t = [sep.tile([128, C], BF16, tag="v", name=f"v{tb}", bufs=NTB)
               for tb in range(NTB)]
        for tb in range(NTB):
            p0, p1_ = tb * 128, min((tb + 1) * 128, T)
            pp = p1_ - p0
            for cg in range(0, C, 512):
                gw = min(512, C - cg)
                p = pt()
                for ci in range(NCH):
                    nc.tensor.matmul(p[:pp, :gw], h[ci][:, p0:p1_],
                                     wqkv[ci][:, 2 * C + cg:2 * C + cg + gw],
                                     start=(ci == 0), stop=(ci == NCH - 1))
                nc.vector.tensor_tensor(v_t[tb][:pp, cg:cg + gw], p[:pp, :gw],
                                        vb_bc[:pp, cg:cg + gw], op=OP.add)

        # =========================================================
        # sparse attention — ordered so the PE stream stays dense and the
        # collective launches right after O2
        # =========================================================
        attnout = [sep.tile([128, T], BF16, tag="ao", name=f"ao{ci}", bufs=6)
                   for ci in range(NCH)]
        for ci in range(NCH):
            # zero pad col 789 (and 788, rewritten by the temporal patch)
            nc.vector.memzero(attnout[ci][:, T - 2:T])

        # kbd (h,j) cols / qbd (j,h) cols, block-diag by head
        kbd = [sm.tile([128, HF], BF16, tag=f"kbd{ci}", name=f"kbd{ci}")
               for ci in range(NCH)]
        qbd = [sm.tile([128, HF], BF16, tag=f"qbd{ci}", name=f"qbd{ci}")
               for ci in range(NCH)]
        for ci in range(NCH):
            nc.vector.memzero(kbd[ci][:])
            nc.vector.memzero(qbd[ci][:])
        for hh in range(H):
            ci, po = hh // 2, (hh % 2) * 64
            nc.vector.tensor_copy(qbd[ci][po:po + 64, hh:hh + 97:H],
                                  q_t[ci][po:po + 64, SPH:SPH + F])
            nc.vector.tensor_copy(kbd[ci][po:po + 64, hh * F:(hh + 1) * F],
                                  k_t[ci][po:po + 64, SPH:SPH + F])

        # rest of qkv: q @ t0
        for j in range(6):
            qk_group(j, *TT[0])

        # vtmp_bd [108, C]: rows (h,j) = temporal v of head h at cols h*64..
        vtmp_bd = sm.tile([HF, C], BF16, tag="vtmpbd", name="vtmpbd")
        nc.vector.memzero(vtmp_bd[0:HF, :])
        for hh in range(H):
            nc.sync.dma_start(vtmp_bd[hh * F:(hh + 1) * F,
                                      hh * 64:(hh + 1) * 64],
                              v_t[6][12:12 + F, hh * 64:(hh + 1) * 64])

        # S1/P1: all local queries vs 9 temporal keys -> p1 [108, T]
        p1 = sm.tile([HF, T], BF16, tag="p1", name="p1")
        for (t0, t1) in TT:
            w = t1 - t0
            p = pt()
            for ci in range(NCH):
                nc.tensor.matmul(p[0:HF, :w], kbd[ci][:], q_t[ci][:, t0:t1],
                                 start=(ci == 0), stop=(ci == NCH - 1))
            nc.scalar.activation(p1[0:HF, t0:t1], p[0:HF, :w], AF.Exp,
                                 scale=SCALE)

        # S2T/P2T: temporal queries vs all local keys, token-major [T, 108]
        p2 = [sm.tile([128, HF], BF16, tag="p2", name=f"p2{tb}", bufs=NTB)
              for tb in range(NTB)]
        for tb in range(NTB):
            p0, p1_ = tb * 128, min((tb + 1) * 128, T)
            pp = p1_ - p0
            p = pt()
            for ci in range(NCH):
                nc.tensor.matmul(p[:pp, 0:HF], k_t[ci][:, p0:p1_], qbd[ci][:],
                                 start=(ci == 0), stop=(ci == NCH - 1))
            nc.scalar.activation(p2[tb][:pp, :], p[:pp, 0:HF], AF.Exp,
                                 scale=SCALE)

        # lsp[h,t] = sum_j p1[(h,j),t]; rlsp = 1/lsp (bf16)
        rlsp = sm.tile([H, T], BF16, tag="rlsp", name="rlsp")
        for (t0, t1) in TT:
            w = t1 - t0
            p = pt()
            nc.tensor.matmul(p[0:H, :w], bd9_t[0:HF, :], p1[0:HF, t0:t1],
                             start=True, stop=True)
            with nc.allow_low_precision(reason="bf16 softmax recip intended"):
                nc.vector.reciprocal(rlsp[0:H, t0:t1], p[0:H, :w])

        # rlsp9 [108, T] = rlsp repeated per j; p1 *= rlsp9 (pre-normalize)
        rlsp9 = sm.tile([HF, T], BF16, tag="rlsp9", name="rlsp9")
        for (t0, t1) in TT:
            w = t1 - t0
            p = pt()
            nc.tensor.matmul(p[0:HF, :w], bd9T_t[0:H, :], rlsp[0:H, t0:t1],
                             start=True, stop=True)
            nc.vector.tensor_copy(rlsp9[0:HF, t0:t1], p[0:HF, :w])

        # mask token-block 6: rows 0-11 (spatial) pass, row 12 (CLS key)
        # kept only for q_j=0 on even cores, rows 13-21 (temporal+pad) zeroed
        nc.vector.tensor_mul(p2[6][0:22, :], p2[6][0:22, :], e00_t[0:22, :])

        # l2 partial [1,108]
        l2row = sm.tile([1, HF], FP32, tag="l2", name="l2row")
        p_l2 = pt()
        for tb in range(NTB):
            p0, p1_ = tb * 128, min((tb + 1) * 128, T)
            pp = p1_ - p0
            nc.tensor.matmul(p_l2[0:1, 0:HF], ones_t[:pp, 0:1],
                             p2[tb][:pp, :],
                             start=(tb == 0), stop=(tb == NTB - 1))
        nc.scalar.copy(l2row[:], p_l2[0:1, 0:HF])

        # normalize p1 per tile (DVE, overlaps O2 matmuls)
        for (t0, t1) in TT:
            nc.vector.tensor_mul(p1[0:HF, t0:t1], p1[0:HF, t0:t1],
                                 rlsp9[0:HF, t0:t1])

        # O2 partial [9, C]
        o2 = sm.tile([F, C], FP32, tag="o2", name="o2")
        for hh in range(H):
            p = pt()
            for tb in range(NTB):
                p0, p1_ = tb * 128, min((tb + 1) * 128, T)
                pp = p1_ - p0
                nc.tensor.matmul(p[0:F, 0:64],
                                 p2[tb][:pp, hh:hh + 97:H],
                                 v_t[tb][:pp, hh * 64:(hh + 1) * 64],
                                 start=(tb == 0), stop=(tb == NTB - 1))
            nc.scalar.copy(o2[0:F, hh * 64:(hh + 1) * 64], p[0:F, 0:64])

        # pairwise AllReduce of (o2 | l2) in one [10, C] buffer — launched
        # as early as possible; consumed in the fused-t0 mid hook
        cc_in = dram.tile([F + 1, C], FP32, tag="cc_in", name="cc_in")
        cc_out = dram.tile([F + 1, C], FP32, tag="cc_out", name="cc_out")
        groups = [[0, 1], [2, 3], [4, 5], [6, 7]]
        nc.scalar.dma_start(cc_in[0:F, :], o2[0:F, :])
        nc.scalar.dma_start(cc_in[F:F + 1, 0:HF], l2row[:])
        nc.gpsimd.collective_compute("AllReduce", mybir.AluOpType.add,
                                     replica_groups=groups,
                                     ins=[cc_in.opt()], outs=[cc_out.opt()])

        # O1: spatial attention out (fills the post-launch PE slot)
        for ci in range(NCH):
            for (t0, t1) in TSP:
                w = t1 - t0
                p = pt()
                nc.tensor.matmul(p[:, :w],
                                 vtmp_bd[0:HF, ci * 128:(ci + 1) * 128],
                                 p1[0:HF, t0:t1], start=True, stop=True)
                nc.vector.tensor_copy(attnout[ci][:, t0:t1], p[:, :w])

        # ---------------- fc2 weights into recycled qkv slots ----------------
        w2g = [wb.tile([128, DFF], BF16, tag="w", name=f"w2g{g}")
               for g in range(NCH)]
        for g in range(NCH):
            for kk in range(4):
                cchunk = 4 * g + kk
                nc.sync.dma_start(w2g[g][:, kk * C:(kk + 1) * C],
                                  fc2Wt[cchunk * 128:(cchunk + 1) * 128, :])

        # =========================================================
        # proj t0 -> LN2 t0 -> fused fc1+fc2 t0 (collective overlapped)
        # =========================================================
        projout = [xfp.tile([128, T], FP32, tag="xf", name=f"po{ci}")
                   for ci in range(NCH)]
        pb = [sep.tile([128, T], BF16, tag="pbb", name=f"pbb{ci}", bufs=6)
              for ci in range(NCH)]
        sq2 = [aa.tile([128, T], BF16, tag="a", name=f"sq2{ci}")
               for ci in range(NCH)]
        h2 = [sep.tile([128, T], BF16, tag="h2", name=f"h2{ci}", bufs=6)
              for ci in range(NCH)]
        bcA2 = sm.tile([128, T], BF16, tag="bc", name="bcA2", bufs=2)
        bcB2 = sm.tile([128, T], BF16, tag="bc", name="bcB2", bufs=2)

        def proj_tile(t0, t1, stats_tags=None):
            # proj couts; optionally interleave LN2 stats accumulation MMs
            # (stats_tags name two free PSUM banks to pin for the sweep)
            w = t1 - t0
            sA = sB = None
            if stats_tags:
                sA, sB = pt(tag=stats_tags[0]), pt(tag=stats_tags[1])
            for j in range(NCH):
                p = pt(tag='mm')
                for ci in range(NCH):
                    nc.tensor.matmul(p[:, :w],
                                     wpj[ci][:, j * 128:(j + 1) * 128],
                                     attnout[ci][:, t0:t1],
                                     start=(ci == 0), stop=(ci == NCH - 1))
                nc.scalar.activation(projout[j][:, t0:t1], p[:, :w],
                                     AF.Identity, bias=pb_t[j][:, 0:1])
                nc.vector.tensor_scalar_add(pb[j][:, t0:t1], p[:, :w],
                                            pb_t[j][:, 0:1])
                nc.vector.tensor_mul(sq2[j][:, t0:t1], pb[j][:, t0:t1],
                                     pb[j][:, t0:t1])
                if stats_tags:
                    nc.tensor.matmul(sA[0:1, :w], ones_t[:, 0:1],
                                     pb[j][:, t0:t1], start=(j == 0),
                                     stop=(j == NCH - 1),
                                     skip_group_check=True)
                    nc.tensor.matmul(sB[0:1, :w], ones_t[:, 0:1],
                                     sq2[j][:, t0:t1], start=(j == 0),
                                     stop=(j == NCH - 1),
                                     skip_group_check=True)
            return sA, sB

        def ln2_finish(sA, sB, t0, t1):
            al2, be2 = ln_chain(sA, sB, t0, t1)
            ln_bcast(al2, be2, bcA2, bcB2, t0, t1)
            ln_apply(projout, h2, bcA2, bcB2, t0, t1)

        def ln2_stats_seq(t0, t1):
            w = t1 - t0
            p = pt(tag='mm')
            for ci in range(NCH):
                nc.tensor.matmul(p[0:1, :w], ones_t[:, 0:1], pb[ci][:, t0:t1],
                                 start=(ci == 0), stop=(ci == NCH - 1))
            p2_ = pt(tag='mm')
            for ci in range(NCH):
                nc.tensor.matmul(p2_[0:1, :w], ones_t[:, 0:1],
                                 sq2[ci][:, t0:t1],
                                 start=(ci == 0), stop=(ci == NCH - 1))
            return p, p2_

        sA0, sB0 = proj_tile(*TT[0], stats_tags=('acc0', 'acc1'))
        ln2_finish(sA0, sB0, *TT[0])

        # fused fc1+fc2: fc2 accumulates into 6 pinned PSUM banks
        def fused(t0, t1, mid_hook=None):
            w = t1 - t0
            acc = [pt(tag=f'acc{cb}') for cb in range(NCH)]
            for g in range(NFF):
                if mid_hook is not None and g == 12:
                    mid_hook()
                pf = pt(tag='mm')
                for ci in range(NCH):
                    nc.tensor.matmul(pf[:, :w],
                                     wf1[ci][:, g * 128:(g + 1) * 128],
                                     h2[ci][:, t0:t1],
                                     start=(ci == 0), stop=(ci == NCH - 1))
                hidt = sm.tile([128, 512], BF16, tag="hid", name=f"hid{g}",
                               bufs=4)
                nc.scalar.activation(hidt[:, :w], pf[:, :w], AF.Gelu,
                                     bias=f1b_t[g][:, 0:1])
                wg, kk = g // 4, g % 4
                for cb in range(NCH):
                    nc.tensor.matmul(acc[cb][:, :w],
                                     w2g[wg][:, kk * C + cb * 128:
                                             kk * C + (cb + 1) * 128],
                                     hidt[:, :w],
                                     start=(g == 0), stop=(g == NFF - 1),
                                     skip_group_check=True)
            for cb in range(NCH):
                st = sm.tile([128, 512], FP32, tag="st", name=f"st{cb}",
                             bufs=4)
                nc.vector.scalar_tensor_tensor(st[:, :w], acc[cb][:, :w],
                                               f2b_t[cb][:, 0:1],
                                               projout[cb][:, t0:t1],
                                               op0=OP.add, op1=OP.add)
                nc.sync.dma_start(outT[cb * 128:(cb + 1) * 128, t0:t1],
                                  st[:, :w])

        # deferred: collective landing -> temporal cols -> t1 of everything
        def temporal_patch():
            l2jh = sm.tile([F, H], FP32, tag="l2jh", name="l2jh")
            o2n = sm.tile([F, C], BF16, tag="o2n", name="o2n")
            # SWDGE cast-DMA f32 -> bf16 straight into o2n
            nc.gpsimd.dma_start(o2n[0:F, :], cc_out[0:F, :])
            # [1,108] DRAM row -> [9,12] SBUF in one reshaping DMA
            nc.scalar.dma_start(l2jh[0:F, :], cc_out[F:F + 1, 0:HF])
            nc.vector.reciprocal(l2jh[0:F, :], l2jh[0:F, :])
            for hh in range(H):
                nc.vector.tensor_scalar_mul(o2n[0:F, hh * 64:(hh + 1) * 64],
                                            o2n[0:F, hh * 64:(hh + 1) * 64],
                                            l2jh[0:F, hh:hh + 1])
            for ci in range(NCH):
                p = ps.tile([128, 512], BF16, tag='mm', bufs=2, name="pstb")
                nc.tensor.transpose(p[:, 0:F],
                                    o2n[0:F, ci * 128:(ci + 1) * 128],
                                    id_t[0:F, 0:F])
                nc.scalar.copy(attnout[ci][:, SPH:SPH + F], p[:, 0:F])
            proj_tile(*TT[1])
            sA1, sB1 = ln2_stats_seq(*TT[1])
            ln2_finish(sA1, sB1, *TT[1])

        fused(*TT[0], mid_hook=temporal_patch)
        fused(*TT[1])

    nc.compile()
    return nc


# ---------------- host side ----------------
_compiled = {}


def kernel(**inputs):
    x = np.ascontiguousarray(np.asarray(inputs['x'], np.float32))
    qkv_w = np.asarray(inputs['qkv_w'], np.float32)
    proj_w = np.asarray(inputs['proj_w'], np.float32)
    proj_b = np.asarray(inputs['proj_b'], np.float32)
    fc1_w = np.asarray(inputs['fc1_w'], np.float32)
    fc1_b = np.asarray(inputs['fc1_b'], np.float32)
    fc2_w = np.asarray(inputs['fc2_w'], np.float32)
    fc2_b = np.asarray(inputs['fc2_b'], np.float32)
    g = np.asarray(inputs['ln2_g'], np.float32)
    bb = np.asarray(inputs['ln2_b'], np.float32)

    import ml_dtypes
    bf16 = ml_dtypes.bfloat16

    # fold LN affine (g, b) into the consuming GEMMs:
    #   W @ (LNraw(x)*g + b) = (W*g) @ LNraw(x) + W@b
    qkvW = qkv_w * g[None, :]                 # [3C, C]
    qkvB = qkv_w @ bb                         # [3C]
    fc1W = fc1_w * g[None, :]
    fc1Bf = fc1_b + fc1_w @ bb

    qkvWt = np.ascontiguousarray(qkvW.T).astype(bf16)     # [C, 3C]
    projWt = np.ascontiguousarray(proj_w.T).astype(bf16)  # [C, C]
    fc1Wt = np.ascontiguousarray(fc1W.T).astype(bf16)     # [C, DFF]
    fc2Wt = np.ascontiguousarray(fc2_w.T).astype(bf16)    # [DFF, C]

    # packed fp32 biases [128, 48]
    auxf_np = np.zeros((128, 48), np.float32)
    for j in range(12):
        auxf_np[:, j] = qkvB[j * 128:(j + 1) * 128]
    for j in range(6):
        auxf_np[:, 12 + j] = proj_b[j * 128:(j + 1) * 128]
    for j in range(24):
        auxf_np[:, 18 + j] = fc1Bf[j * 128:(j + 1) * 128]
    for j in range(6):
        auxf_np[:, 42 + j] = fc2_b[j * 128:(j + 1) * 128]

    # packed bf16 constants [128, 1253]:
    # ones(0) | ident(1:129) | e00(129:237) | bd9(237:249) |
    # onesrow(249:377) | bd9T(377:485) | vbrow(485:1253)
    bd9_np = np.zeros((H * F, H), np.float32)
    for hh in range(H):
        bd9_np[hh * F:(hh + 1) * F, hh] = 1.0
    auxb_np = np.zeros((128, 1253), np.float32)
    auxb_np[:, 0] = 1.0
    auxb_np[:, 1:129] = np.eye(128)
    # e00: multiplicative mask for p2 token-block 6 (local tokens 768..789):
    # rows 0-11 = spatial -> 1; row 12 = CLS key -> keep only q_j=0 cols
    # (cols 0..11 in (j,h) order) on even cores; rows 13-21 -> 0
    auxb_np[0:12, 129:237] = 1.0
    auxb_np[0:108, 237:249] = bd9_np
    auxb_np[0, 249:377] = 1.0
    auxb_np[0:12, 377:485] = bd9_np.T
    auxb_np[0, 485:1253] = qkvB[2 * C:]
    auxb_even = auxb_np.copy()
    auxb_even[12, 129 + 0:129 + H] = 1.0    # CLS self-term on even cores

    in_maps = []
    for core in range(8):
        b_, half = core // 2, core % 2
        sp = x[b_, F + half * SPH: F + (half + 1) * SPH]     # [780, C]
        tmp = x[b_, 0:F]                                     # [9, C]
        pad = np.zeros((1, C), np.float32)
        xTn = np.ascontiguousarray(
            np.concatenate([sp, tmp, pad], 0).T).astype(bf16)  # [C, 790]
        in_maps.append(dict(
            xT=xTn, qkvWt=qkvWt, projWt=projWt, fc1Wt=fc1Wt, fc2Wt=fc2Wt,
            auxf=auxf_np,
            auxb=(auxb_even if half == 0 else auxb_np).astype(bf16)))

    if 'nc' not in _compiled:
        _compiled['nc'] = build_kernel()
    nc = _compiled['nc']
    res = run_bass_kernel_spmd(nc, in_maps, list(range(8)))
    _compiled['last_result'] = res

    out = np.zeros((B, N, C), np.float32)
    for core in range(8):
        b_, half = core // 2, core % 2
        oT = res.results[core]['outT']                       # [C, 790]
        if half == 0:
            out[b_, 0:F] = oT[:, SPH:SPH + F].T
            out[b_, F:F + SPH] = oT[:, 0:SPH].T
        else:
            out[b_, F + SPH:N] = oT[:, 0:SPH].T
    return out


if __name__ == '__main__':
    from reference import setup_inputs, reference
    inputs = {k: np.asarray(v) for k, v in setup_inputs().items()}
    out = kernel(**inputs)
    print("kernel ran, out shape", out.shape)
